# revision 23
# baseline (speedup 1.0000x reference)
"""Trainium2 Bass kernel for the SE + patch-correlation-attention + down-conv module.

Sharding (8 cores): split the 96 image rows into 8 slabs of 12 rows. Each core:
  1. SE gate from host-precomputed global channel sums (tiny MLP on device)
  2. FF = x*y, S = sigmoid(x*y) maps on its 14-row halo slab (bf16)
  3. pipelined patch-correlation attention: shifts processed in 3 batches of 3;
     per batch: DVE products + PE one-hot channel-reduction -> A rows, exp on
     ACT, DMA partition-broadcast of exp rows, then DVE products + PE identity
     accumulation for the weighted sum. Softmax denominator via PE ones-reduce
     + reciprocal + broadcast at the end.
  4. AllToAll (bf16) redistributes the attention output so core k holds
     the 32-column slice [32k,32k+32) of the .view()-scrambled Z2 matrix
  5. local 256x256 down matmul, InstanceNorm partials -> tiny AllReduce,
     normalize + LeakyReLU, write its (256, 32, 36) output slice
Host gathers the 8 slices and permutes into (1, 256, 96, 96).

A dummy 8-byte AllReduce is triggered first so the one-time CC bootstrap
barrier overlaps the compute instead of serializing before the AllToAll.
"""
import numpy as np

C, H, W, M = 256, 96, 96, 8
RPC = H // M          # 12 rows per core
P = RPC * W           # 1152 positions per core
SLAB = RPC + 2        # 14 rows incl. halo
WP = 100              # padded slab width (even stride, j0 at col 2)
HW = H * W            # 9216
SHIFTS = [(di, dj) for di in (-1, 0, 1) for dj in (-1, 0, 1)]
CHUNKS = [(0, 512), (512, 512), (1024, 128)]   # psum-bank-aligned matmul N-chunks

_cache = {}


def _build():
    import concourse.bass as bass
    from concourse import bacc
    import concourse.mybir as mybir
    from concourse.tile import TileContext
    from concourse.masks import make_identity

    fp32 = mybir.dt.float32
    bf16 = mybir.dt.bfloat16
    AF = mybir.ActivationFunctionType
    Alu = mybir.AluOpType
    GROUPS = [list(range(M))]

    nc = bacc.Bacc()

    xs = nc.declare_dram_parameter("xs", [C, SLAB, W], bf16, isOutput=False)
    yg = nc.declare_dram_parameter("yg", [128, 2], fp32, isOutput=False)
    msk = nc.declare_dram_parameter("msk", [128, 2], fp32, isOutput=False)
    dwt = nc.declare_dram_parameter("dwt", [C, C], bf16, isOutput=False)
    outp = nc.declare_dram_parameter("out", [C, 32, 36], fp32, isOutput=True)

    dum_in = nc.dram_tensor("dum_in", [1, 2], fp32)
    dum_out = nc.dram_tensor("dum_out", [1, 2], fp32, addr_space="Shared")
    a2a_in = nc.dram_tensor("a2a_in", [M, 32, P], bf16)
    a2a_out = nc.dram_tensor("a2a_out", [M, 32, P], bf16)
    st_part = nc.dram_tensor("st_part", [128, 4], fp32)
    st_sum = nc.dram_tensor("st_sum", [128, 4], fp32, addr_space="Shared")

    a_dram = nc.dram_tensor("a_dram", [9, P], bf16)
    r_dram = nc.dram_tensor("r_dram", [P], fp32)
    dma = nc.default_dma_engine

    with TileContext(nc) as tc:
        with (
            tc.tile_pool(name="const", bufs=1) as cp,
            tc.tile_pool(name="sb", bufs=1) as sp,
            tc.tile_pool(name="work", bufs=6) as wp,
        ):
            # ---------- small parameter loads first ----------
            y_sb = sp.tile([128, 2], fp32, tag="ygate")
            dma.dma_start(out=y_sb, in_=yg[:, :])
            msk_sb = cp.tile([128, 2], fp32)
            dma.dma_start(out=msk_sb, in_=msk[:, :])
            dw_sb = [cp.tile([128, C], bf16, tag=f"dw_{ct}", name=f"dw_{ct}") for ct in range(2)]
            for ct in range(2):
                nc.gpsimd.dma_start(out=dw_sb[ct], in_=dwt[128 * ct : 128 * ct + 128, :])

            # ---------- load x slab (bf16) ----------
            x_sb = [sp.tile([128, SLAB, W], bf16, tag=f"x{ct}", name=f"x{ct}") for ct in range(2)]
            for ct in range(2):
                dma.dma_start(out=x_sb[ct][:, 0:7, :], in_=xs[128 * ct : 128 * ct + 128, 0:7, :])
                dma.dma_start(out=x_sb[ct][:, 7:14, :], in_=xs[128 * ct : 128 * ct + 128, 7:14, :])

            # ---------- pre-warm the ACT tables the head needs ----------
            warm_in = cp.tile([1, 2], fp32)
            warm_out = cp.tile([1, 2], fp32)
            nc.vector.memset(warm_in, 0.25)
            nc.scalar.activation(out=warm_out, in_=warm_in, func=AF.Sigmoid)

            # ---------- constants ----------
            ident = cp.tile([128, 128], bf16)
            make_identity(nc, ident)
            # e3[:, j, :]: one-hot column j (selects psum row j of the A batch)
            e3 = cp.tile([128, 3, 3], bf16)
            nc.gpsimd.memset(e3, 0.0)
            for j in range(3):
                nc.gpsimd.memset(e3[:, j, j : j + 1], 1.0)
            ones6 = cp.tile([67, 1], bf16)
            nc.gpsimd.memset(ones6, 1.0)
            eps_sb = cp.tile([128, 1], fp32)
            nc.gpsimd.memset(eps_sb, 1e-5)

            # ---------- FF and S maps (bf16, packed ct, zero-padded 14x100) ----
            # main copy: image col j at slab col 2+j; "+1" copy: j at 3+j so
            # dj=+-1 shifted views stay 4B-aligned for DVE 2x.
            ff_sb = sp.tile([128, 2, SLAB, WP], bf16, tag="ff")
            s_sb = sp.tile([128, 2, SLAB, WP], bf16, tag="s")
            ff2_sb = sp.tile([128, 2, SLAB, WP], bf16, tag="ff2")
            s2_sb = sp.tile([128, 2, SLAB, WP], bf16, tag="s2")
            # only the pad columns the dj=+-1 views read need zeroing
            nc.gpsimd.memset(s2_sb[:, :, :, 2:3], 0.0)
            nc.gpsimd.memset(s2_sb[:, :, :, 99:100], 0.0)
            nc.gpsimd.memset(ff2_sb[:, :, :, 2:3], 0.0)
            nc.gpsimd.memset(ff2_sb[:, :, :, 99:100], 0.0)
            for ct in range(2):
                nc.vector.tensor_scalar(
                    out=ff_sb[:, ct, :, 2:98], in0=x_sb[ct],
                    scalar1=y_sb[:, ct : ct + 1], scalar2=None, op0=Alu.mult,
                )
                nc.scalar.activation(
                    out=s_sb[:, ct, :, 2:98], in_=x_sb[ct],
                    func=AF.Sigmoid, scale=y_sb[:, ct : ct + 1],
                )
            # warm the Exp table before the first softmax batch needs it
            nc.scalar.activation(out=warm_out, in_=warm_in, func=AF.Exp)
            # zero invalid halo rows of S (top/bottom image edge)
            nc.vector.tensor_scalar(
                out=s_sb[:, :, 0, 2:98], in0=s_sb[:, :, 0, 2:98],
                scalar1=msk_sb[:, 0:1], scalar2=None, op0=Alu.mult,
            )
            nc.vector.tensor_scalar(
                out=s_sb[:, :, 13, 2:98], in0=s_sb[:, :, 13, 2:98],
                scalar1=msk_sb[:, 1:2], scalar2=None, op0=Alu.mult,
            )
            nc.vector.tensor_copy(out=s2_sb[:, :, :, 3:99], in_=s_sb[:, :, :, 2:98])

            def sview(di, dj):
                """4B-aligned packed-ct view of S shifted by (di, dj)."""
                if dj == 0:
                    return s_sb[:, :, 1 + di : 13 + di, 2:98]
                return s2_sb[:, :, 1 + di : 13 + di, 3 + dj : 99 + dj]

            def ffview(di, dj):
                if dj == 0:
                    return ff_sb[:, :, 1 + di : 13 + di, 2:98]
                return ff2_sb[:, :, 1 + di : 13 + di, 3 + dj : 99 + dj]

            # ---------- pipelined attention ----------
            # exp / A double-buffer via partition-offset views (batch b at
            # partitions 32*(b%2)..+3) so phase2(b+1) never waits on exp(b).
            exp_b = [sp.tile([3, P], bf16, tag=f"exp{b}", name=f"exp{b}") for b in range(3)]
            rep_sb = [sp.tile([128, P], bf16, tag=f"rep{d}", name=f"rep{d}") for d in range(9)]
            oat = [sp.tile([128, P], bf16, tag=f"oat{ct}", name=f"oat{ct}") for ct in range(2)]
            # acc holds both ct halves: ct0 cols [0,1152) in psum banks 0-2,
            # ct1 cols [1152,2304) in banks 2-4; per-ct chunk splits below
            # keep every matmul output inside a single psum bank.
            CH1 = [(0, 384), (384, 512), (896, 256)]

            def phase2(b):
                A_b = pA.tile([3, P], fp32, tag="Ab", name="Ab")
                for j in range(3):
                    di, dj = SHIFTS[3 * b + j]
                    prod = wp.tile([128, 2, P], bf16, tag="prod")
                    nc.vector.tensor_tensor(
                        out=prod.rearrange("c k (r w) -> c k r w", w=W),
                        in0=s_sb[:, :, 1:13, 2:98],
                        in1=sview(di, dj), op=Alu.mult,
                    )
                    for ct in range(2):
                        for (o, n) in CHUNKS:
                            nc.tensor.matmul(
                                A_b[:, o : o + n], e3[:, j, :], prod[:, ct, o : o + n],
                                start=(j == 0 and ct == 0), stop=(j == 2 and ct == 1),
                            )
                # softmax numerator for this batch; bounce via DRAM for the
                # partition-broadcast source AP (gpsimd SWDGE queue).
                nc.scalar.activation(out=exp_b[b], in_=A_b,
                                     func=AF.Exp, scale=1.0 / C)
                nc.gpsimd.dma_start(out=a_dram[3 * b : 3 * b + 3, :], in_=exp_b[b])
                for j in range(3):
                    d = 3 * b + j
                    nc.gpsimd.dma_start(
                        out=rep_sb[d], in_=a_dram[d, :].partition_broadcast(128),
                    )

            def phase3(b):
                for j in range(3):
                    d = 3 * b + j
                    di, dj = SHIFTS[d]
                    prod = wp.tile([128, 2, P], bf16, tag="prod")
                    rv = rep_sb[d].rearrange("c (r w) -> c r w", w=W)
                    nc.vector.tensor_tensor(
                        out=prod.rearrange("c k (r w) -> c k r w", w=W),
                        in0=ffview(di, dj),
                        in1=rv.unsqueeze(1).broadcast_to([128, 2, RPC, W]),
                        op=Alu.mult,
                    )
                    for ct in range(2):
                        for (o, n) in (CHUNKS if ct == 0 else CH1):
                            # chunks living in the shared psum bank 2 (acc
                            # cols 1024..1536) accumulate onto the explicit
                            # memset instead of using start-zeroing, which
                            # clears the whole bank.
                            shared = (ct == 0 and o == 1024) or (ct == 1 and o == 0)
                            nc.tensor.matmul(
                                acc[:, ct * P + o : ct * P + o + n],
                                ident, prod[:, ct, o : o + n],
                                start=(d == 0 and not shared), stop=(d == 8),
                                skip_group_check=shared,
                            )

            with tc.tile_pool(name="ps_acc", bufs=1, space="PSUM") as pa:
                acc = pa.tile([128, 2 * P], fp32, tag="acc")
                # bank 2 of acc (cols 1024..1536) is shared between the ct0
                # and ct1 accumulation groups; zero it once up front.
                nc.vector.memset(acc[:, 1024:1536], 0.0)
                with tc.tile_pool(name="ps_A", bufs=1, space="PSUM") as pA:
                    # software-pipelined emission: phase2(b+1) before
                    # phase3(b) so the DVE never stalls on exp/broadcast.
                    phase2(0)
                    # FF2 shifted copy is first needed by phase3(0)
                    nc.vector.tensor_copy(out=ff2_sb[:, :, :, 3:99],
                                          in_=ff_sb[:, :, :, 2:98])
                    phase2(1)
                    phase3(0)
                    phase2(2)
                    phase3(1)
                    phase3(2)

                # warm the Sqrt/Prelu tables for the tail while ACT is idle
                nc.scalar.activation(out=warm_out, in_=warm_in, func=AF.Sqrt)
                nc.scalar.activation(out=warm_out, in_=warm_in, func=AF.Prelu, alpha=0.2)

                # ---------- softmax denominator + reciprocal + broadcast ----
                with tc.tile_pool(name="ps_den", bufs=1, space="PSUM") as pd:
                    den_ps = pd.tile([1, P], fp32, tag="den")
                    for b in range(3):
                        for (o, n) in CHUNKS:
                            nc.tensor.matmul(den_ps[:, o : o + n],
                                             ones6[0:3, :],
                                             exp_b[b][:, o : o + n],
                                             start=(b == 0), stop=(b == 2))
                    rec_row = sp.tile([1, P], fp32, tag="recrow")
                    nc.vector.reciprocal_approx_fast(out=rec_row, in_=den_ps)
                nc.gpsimd.dma_start(out=r_dram[:], in_=rec_row)
                rec_rep = sp.tile([128, P], fp32, tag="recrep")
                nc.gpsimd.dma_start(out=rec_rep, in_=r_dram[:].partition_broadcast(128))

                for ct in range(2):
                    nc.vector.tensor_tensor(out=oat[ct], in0=acc[:, ct * P : ct * P + P],
                                            in1=rec_rep, op=Alu.mult)
                    nc.gpsimd.dma_start(out=a2a_in[4 * ct : 4 * ct + 4, :, :], in_=oat[ct])

            # ---------- AllToAll ----------
            nc.gpsimd.collective_compute(
                "AllToAll", Alu.bypass, replica_groups=GROUPS,
                ins=[a2a_in[:, :, :]], outs=[a2a_out[:, :, :]],
            )

            # ---------- down matmul on the scrambled layout ----------
            # rhs[ch, s, t] = a2a_out[ch//32, s, 36*(ch%32)+t]
            rhs_sb = [sp.tile([128, 32, 36], bf16, tag=f"rhs{kt}", name=f"rhs{kt}") for kt in range(2)]
            v = a2a_out.rearrange("j s (b t) -> j b s t", t=36)
            ENGS = [dma, nc.scalar, nc.gpsimd]
            for kt in range(2):
                for a in range(4):
                    ENGS[(4 * kt + a) % 3].dma_start(
                        out=rhs_sb[kt][32 * a : 32 * a + 32, :, :],
                        in_=v[4 * kt + a],
                    )

            stat_sb = sp.tile([128, 4], fp32, tag="stat")
            sq_scr = wp.tile([128, P], fp32, tag="sqscr")
            zo_sb = [sp.tile([128, P], fp32, tag=f"zo{mt}", name=f"zo{mt}") for mt in range(2)]
            with tc.tile_pool(name="ps_z", bufs=1, space="PSUM") as pz:
                z_ps = [pz.tile([128, P], fp32, tag=f"z{mt}", name=f"z{mt}") for mt in range(2)]
                for mt in range(2):
                    for (o, n) in CHUNKS:
                        for kt in range(2):
                            nc.tensor.matmul(
                                z_ps[mt][:, o : o + n],
                                dw_sb[kt][:, 128 * mt : 128 * mt + 128],
                                rhs_sb[kt].rearrange("c s t -> c (s t)")[:, o : o + n],
                                start=(kt == 0), stop=(kt == 1),
                            )
                    # IN stats partials
                    nc.vector.tensor_reduce(
                        out=stat_sb[:, mt : mt + 1], in_=z_ps[mt],
                        axis=mybir.AxisListType.X, op=Alu.add,
                    )
                    nc.scalar.activation(
                        out=sq_scr, in_=z_ps[mt], func=AF.Square,
                        accum_out=stat_sb[:, 2 + mt : 3 + mt],
                    )
                nc.gpsimd.dma_start(out=st_part[:, :], in_=stat_sb)
                nc.gpsimd.collective_compute(
                    "AllReduce", Alu.add, replica_groups=GROUPS,
                    ins=[st_part[:, :]], outs=[st_sum[:, :]],
                )
                gl_sb = sp.tile([128, 4], fp32, tag="glstat")
                nc.gpsimd.dma_start(out=gl_sb, in_=st_sum[:, :])

                # mu = sum/HW ; var = sumsq/HW - mu^2 ; inv = 1/sqrt(var+eps)
                ins_sb = sp.tile([128, 8], fp32, tag="instat")
                mu2 = ins_sb[:, 0:2]
                e22 = ins_sb[:, 2:4]
                inv2 = ins_sb[:, 4:6]
                nmi2 = ins_sb[:, 6:8]
                nc.vector.tensor_scalar(out=mu2, in0=gl_sb[:, 0:2],
                                        scalar1=1.0 / HW, scalar2=None, op0=Alu.mult)
                nc.vector.tensor_scalar(out=e22, in0=gl_sb[:, 2:4],
                                        scalar1=1.0 / HW, scalar2=None, op0=Alu.mult)
                nc.vector.tensor_tensor(out=inv2, in0=mu2, in1=mu2, op=Alu.mult)
                nc.vector.tensor_tensor(out=e22, in0=e22, in1=inv2, op=Alu.subtract)
                nc.scalar.activation(out=e22, in_=e22, func=AF.Sqrt, bias=eps_sb, scale=1.0)
                nc.vector.reciprocal(out=inv2, in_=e22)
                # nmi = -mu * inv  (bias for the fused Prelu normalize)
                nc.vector.scalar_tensor_tensor(out=nmi2, in0=mu2, scalar=-1.0,
                                               in1=inv2, op0=Alu.mult, op1=Alu.mult)
                for mt in range(2):
                    # LeakyReLU((z - mu) * inv) fused on ScalarE:
                    #   prelu(z*inv + (-mu*inv), alpha=0.2)
                    nc.scalar.activation(
                        out=zo_sb[mt], in_=z_ps[mt], func=AF.Prelu,
                        bias=ins_sb[:, 6 + mt : 7 + mt],
                        scale=ins_sb[:, 4 + mt : 5 + mt], alpha=0.2,
                    )
                    eng = dma if mt == 0 else nc.scalar
                    eng.dma_start(
                        out=outp[128 * mt : 128 * mt + 128, :, :],
                        in_=zo_sb[mt].rearrange("c (s t) -> c s t", t=36),
                    )
    nc.compile()
    return nc


def _get_nc():
    if "nc" not in _cache:
        _cache["nc"] = _build()
    return _cache["nc"]


def _shard_inputs(x, se_w1, se_b1, se_w2, se_b2, down_w):
    import ml_dtypes

    x = np.ascontiguousarray(np.asarray(x, np.float32))[0]          # (C, H, W)
    mean = x.mean(axis=(1, 2), dtype=np.float32)                    # (C,)
    h = np.maximum(np.asarray(se_w1, np.float32) @ mean
                   + np.asarray(se_b1, np.float32), 0.0)
    ylog = np.asarray(se_w2, np.float32) @ h + np.asarray(se_b2, np.float32)
    y = (1.0 / (1.0 + np.exp(-ylog))).astype(np.float32)
    yg = np.ascontiguousarray(y.reshape(2, 128).T)                  # (128, 2)
    dwt = np.ascontiguousarray(
        np.asarray(down_w, np.float32).T.astype(ml_dtypes.bfloat16)
    )                                                               # (C, C) bf16

    in_maps = []
    for k in range(M):
        slab = np.zeros((C, SLAB, W), np.float32)
        lo, hi = RPC * k - 1, RPC * k + RPC + 1
        clo, chi = max(lo, 0), min(hi, H)
        slab[:, clo - lo : clo - lo + (chi - clo), :] = x[:, clo:chi, :]
        msk = np.ones((128, 2), np.float32)
        if k == 0:
            msk[:, 0] = 0.0
        if k == M - 1:
            msk[:, 1] = 0.0
        in_maps.append({
            "xs": slab.astype(ml_dtypes.bfloat16), "yg": yg, "msk": msk,
            "dwt": dwt,
        })
    return in_maps


def _gather(results):
    R = np.stack([np.asarray(r["out"], np.float32) for r in results])  # (8, 256, 32, 36)
    return np.ascontiguousarray(
        R.transpose(1, 3, 0, 2).reshape(1, C, H, W).astype(np.float32)
    )


def kernel(x, se_w1, se_b1, se_w2, se_b2, down_w, _trace=False):
    from concourse.bass_utils import run_bass_kernel_spmd

    nc = _get_nc()
    in_maps = _shard_inputs(x, se_w1, se_b1, se_w2, se_b2, down_w)
    res = run_bass_kernel_spmd(nc, in_maps, core_ids=list(range(M)), trace=_trace)
    out = _gather(res.results)
    if _trace:
        kernel.last_results = res
    return out
